# revision 12
# baseline (speedup 1.0000x reference)
# Trainium2 Bass kernel for nn_MultiHeadAttention_57363583205864
# DeBERTa-style disentangled attention (c2c + c2p + p2c), 8-core head-sharded.
#
# Sharding: core c owns heads {2c, 2c+1} for both batches (128 of 1024 concat
# dims). Each core computes its heads' projections, attention, and a partial
# output projection; the host sums the 8 partials and adds bc.
#
# Device-side layout notes:
#  - All matmuls run in float32r (full PE rate, ~1.5e-4 rel err).
#  - Scores are built TRANSPOSED: scoresT[j, i] so softmax Z comes free via a
#    ones-row in the OV matmul and attn@v needs no transposes.
#  - c2p gather: CPrev[i, m'] = q'_i . kr_{1023-m'} (reversal folded into the
#    matmul rhs AP), padded per i-tile, read with a diagonal SBUF->SBUF DMA in
#    normal [i, j] layout, then PE-transposed into the score PSUM accumulation.
#  - p2c gather: BT[j, m] = k_j . qr'_m, padded per j-tile, read with a
#    diagonal DMA directly in [j, i] layout, added on DVE before the exp.
import sys

sys.path.insert(0, "/opt/trn_rl_repo")

import numpy as np

B, S, D, H, DH = 2, 1024, 1024, 16, 64
T = B * S
NCORES = 8
SCALE = float(np.sqrt(3 * DH))
PADW = 128            # clamp pad on each side of the 1024-wide relative index
BLKW = PADW + S + PADW  # 1536 per tile-block
WTOT = 8 * BLKW       # CPP/BTP free width (12288)
NEG = -1e30

_cached = {}
STAGE = 4  # 1=phaseA 2=+CP/BT 3=+scores/OV 4=full


def _build_nc():
    import concourse.bass as bass
    import concourse.bacc as bacc
    import concourse.mybir as mybir
    from concourse.tile import TileContext
    from concourse.ap import AP

    f32 = mybir.dt.float32
    f32r = mybir.dt.float32r
    Ident = mybir.ActivationFunctionType.Identity
    Exp = mybir.ActivationFunctionType.Exp
    Add = mybir.AluOpType.add
    Mult = mybir.AluOpType.mult

    nc = bacc.Bacc("TRN2", target_bir_lowering=False, debug=False,
                   num_devices=NCORES)

    xr = nc.dram_tensor("xr", [T, D], f32r, kind="ExternalInput")
    rr = nc.dram_tensor("rr", [T, D], f32r, kind="ExternalInput")
    w_in = {}
    for wn in ("wq", "wk", "wv", "wqr", "wkr"):
        w_in[wn] = nc.dram_tensor(wn, [D, 128], f32r, kind="ExternalInput")
    wc = nc.dram_tensor("wc", [128, D], f32r, kind="ExternalInput")
    b_in = {}
    for bn in ("bq", "bk", "bv", "bqr", "bkr"):
        b_in[bn] = nc.dram_tensor(bn, [128, 1], f32, kind="ExternalInput")
    mb = nc.dram_tensor("mb", [128, 16], f32, kind="ExternalInput")
    outT = nc.dram_tensor("outT", [D, T], f32, kind="ExternalOutput")

    ident_np = np.eye(128, dtype=np.float32)
    oh_np = np.zeros((65, 256), dtype=np.float32)
    oh_np[64, 0:64] = 1.0
    oh_np[64, 192:256] = 1.0
    ones_np = np.ones((128, 32), dtype=np.float32)

    with TileContext(nc) as tc:
        ident_dram = nc.inline_tensor(ident_np, name="identc")
        oh_dram = nc.inline_tensor(oh_np, name="ohc")
        ones_dram = nc.inline_tensor(ones_np, name="onesc")

        with tc.tile_pool(name="pers", bufs=1) as pers, \
             tc.tile_pool(name="bigb", bufs=2) as bigb, \
             tc.tile_pool(name="strips", bufs=2) as strips, \
             tc.tile_pool(name="misc", bufs=1) as misc:

            identR = pers.tile([128, 128], f32r, tag="ident")
            nc.sync.dma_start(identR[:], ident_dram.ap().bitcast(f32r))
            ohS = pers.tile([65, 256], f32r, tag="ohS")
            nc.sync.dma_start(ohS[:], oh_dram.ap().bitcast(f32r))
            mbS = pers.tile([128, 16], f32, tag="mb")
            nc.sync.dma_start(mbS[:], mb.ap())

            projT = {}
            for pn in ("qT", "kT", "qrT", "krT", "vT"):
                projT[pn] = pers.tile([128, T], f32r, tag=pn, name=pn)
            vext = pers.tile([128, 32 * 65], f32r, tag="vext")
            nc.sync.dma_start(
                AP(vext[:].tensor, 64, [[32 * 65, 128], [65, 32], [1, 1]]),
                AP(ones_dram.ap().bitcast(f32r).tensor, 0, [[32, 128], [1, 32], [1, 1]]))
            wcS = pers.tile([128, D], f32r, tag="wcS")
            nc.sync.dma_start(wcS[:], wc.ap())

            # ---------------- Phase A: x.T/rel.T transposes + projections ---
            with tc.tile_pool(name="wS", bufs=1) as wsp, \
                 tc.tile_pool(name="sbA", bufs=2) as sbA, \
                 tc.tile_pool(name="psA", bufs=6, space="PSUM") as psA, \
                 tc.tile_pool(name="psT", bufs=2, space="PSUM") as psT:
                wtiles = {}
                for wn in ("wq", "wk", "wv", "wqr", "wkr"):
                    wt = wsp.tile([128, 8, 128], f32r, tag=wn)
                    nc.sync.dma_start(
                        wt[:], w_in[wn].ap().rearrange("(a p) d -> p a d", p=128))
                    wtiles[wn] = wt
                bS = wsp.tile([128, 8], f32, tag="bS")
                btiles = {}
                for bi, bn in enumerate(("bq", "bk", "bv", "bqr", "bkr")):
                    nc.sync.dma_start(bS[:, bi:bi + 1], b_in[bn].ap())
                    btiles[bn] = bS[:, bi:bi + 1]

                PROJ = [("qT", "wq", "bq", "x"), ("kT", "wk", "bk", "x"),
                        ("vT", "wv", "bv", "x"), ("qrT", "wqr", "bqr", "r"),
                        ("krT", "wkr", "bkr", "r")]
                for ch in range(4):  # 512-token chunks
                    t0 = ch * 512
                    xb = bigb.tile([128, 4, D], f32r, tag="big")
                    rb = bigb.tile([128, 4, D], f32r, tag="big")
                    nc.sync.dma_start(
                        xb[:], xr.ap()[t0:t0 + 512, :].rearrange("(a p) d -> p a d", p=128))
                    nc.sync.dma_start(
                        rb[:], rr.ap()[t0:t0 + 512, :].rearrange("(a p) d -> p a d", p=128))
                    pj = {}
                    for pn, _, _, _ in PROJ:
                        pj[pn] = psA.tile([128, 512], f32, tag="pj", name=f"pj_{pn}_{ch}")
                    for kt in range(8):
                        xTk = sbA.tile([128, 512], f32r, tag="xT")
                        rTk = sbA.tile([128, 512], f32r, tag="rT")
                        for src, dst in ((xb, "x"), (rb, "r")):
                            tp = psT.tile([128, 512], f32, tag="tp")
                            for s in range(4):
                                nc.tensor.matmul(
                                    tp[:, s * 128:(s + 1) * 128].bitcast(f32r),
                                    src[:, s, kt * 128:(kt + 1) * 128],
                                    identR[:], is_transpose=True,
                                    start=True, stop=True)
                            tgt = xTk if dst == "x" else rTk
                            nc.scalar.copy(tgt[:], tp[:])
                        for pn, wn, bn, sn in PROJ:
                            nc.tensor.matmul(
                                pj[pn][:], wtiles[wn][:, kt, :],
                                xTk[:] if sn == "x" else rTk[:],
                                start=(kt == 0), stop=(kt == 7))
                    for pn, wn, bn, sn in PROJ:
                        nc.scalar.activation(
                            projT[pn][:, t0:t0 + 512], pj[pn][:], Ident,
                            bias=btiles[bn])

                # v transposes -> vext tiles (both heads per 128-block)
                for b in range(B):
                    for jt in range(8):
                        tp = psT.tile([128, 512], f32, tag="tp")
                        nc.tensor.matmul(
                            tp[:, 0:128].bitcast(f32r),
                            projT["vT"][:, b * S + jt * 128: b * S + (jt + 1) * 128],
                            identR[:], is_transpose=True, start=True, stop=True)
                        for hl in range(2):
                            vsl = vext[:, ((b * 2 + hl) * 8 + jt) * 65:
                                       ((b * 2 + hl) * 8 + jt) * 65 + 65]
                            nc.scalar.copy(vsl[:, 0:64], tp[:, hl * 64:hl * 64 + 64])

            # ---------------- Phase B: attention per (batch, head) ----------
            BATCHES = range(B) if STAGE >= 2 else range(0)
            psS = ctx_psS = tc.tile_pool(name="psS", bufs=2, space="PSUM")
            psO = ctx_psO = tc.tile_pool(name="psO", bufs=2, space="PSUM")
            psS = ctx_psS.__enter__()
            psO = ctx_psO.__enter__()
            for b in BATCHES:
                zrS = misc.tile([65, 2 * S], f32r, tag="zrS", name=f"zrS_{b}")
                OTn = misc.tile([128, S], f32r, tag="OTn", name=f"OTn_{b}")
                OT = misc.tile([128, S], f32, tag="OTb", name=f"OT_{b}")
                for hl in range(2):
                    rg = hl * 64
                    hcols = slice(rg, rg + 64)
                    bcols = slice(b * S, (b + 1) * S)
                    CPP = bigb.tile([128, WTOT], f32r, tag="big")
                    BTP = bigb.tile([128, WTOT], f32r, tag="big")
                    krT_rev = projT["krT"][hcols, bcols][:, ::-1]
                    for t8 in range(8):
                        pcp = psS.tile([128, 1024], f32, tag="cpbt")
                        pbt = psS.tile([128, 1024], f32, tag="cpbt")
                        isl = slice(t8 * 128, (t8 + 1) * 128)
                        for mc in range(2):
                            msl = slice(mc * 512, (mc + 1) * 512)
                            nc.tensor.matmul(
                                pcp[:, msl],
                                projT["qT"][hcols, bcols][:, isl],
                                krT_rev[:, msl],
                                start=True, stop=True, tile_position=(rg, 0))
                            nc.tensor.matmul(
                                pbt[:, msl],
                                projT["kT"][hcols, bcols][:, isl],
                                projT["qrT"][hcols, bcols][:, msl],
                                start=True, stop=True, tile_position=(rg, 0))
                        for buf, ps in ((CPP, pcp), (BTP, pbt)):
                            base = t8 * BLKW
                            nc.scalar.copy(buf[:, base + PADW:base + PADW + S], ps[:])
                            nc.vector.tensor_copy(
                                buf[:, base:base + PADW],
                                buf[:, base + PADW:base + PADW + 1].broadcast_to([128, PADW]))
                            nc.vector.tensor_copy(
                                buf[:, base + PADW + S:base + BLKW],
                                buf[:, base + PADW + S - 1:base + PADW + S].broadcast_to([128, PADW]))

                    vbase = (b * 2 + hl) * 8
                    for ch in (range(2) if STAGE >= 3 else range(0)):
                        ic0 = ch * 512
                        pov = psO.tile([128, 512], f32, tag="ov")
                        for jt in range(8):
                            psc = psS.tile([128, 512], f32, tag="sc")
                            # c2cT = k . q'
                            nc.tensor.matmul(
                                psc[:],
                                projT["kT"][hcols, bcols][:, jt * 128:(jt + 1) * 128],
                                projT["qT"][hcols, bcols][:, ic0:ic0 + 512],
                                start=True, stop=False, tile_position=(rg, 0))
                            # c2p tiles: diagonal DMA from CPP then PE transpose-acc
                            cstrip = strips.tile([128, 512], f32r, tag="cstrip")
                            pstrip = strips.tile([128, 512], f32r, tag="pstrip")
                            for s in range(4):
                                it = ch * 4 + s
                                dlt = jt - it
                                if dlt <= -5:      # fully low-clamped (m'=0 edge)
                                    nc.vector.tensor_copy(
                                        cstrip[:, s * 128:(s + 1) * 128],
                                        CPP[:, it * BLKW + PADW:it * BLKW + PADW + 1]
                                        .broadcast_to([128, 128]))
                                elif dlt >= 5:     # fully high-clamped
                                    nc.vector.tensor_copy(
                                        cstrip[:, s * 128:(s + 1) * 128],
                                        CPP[:, it * BLKW + PADW + S - 1:it * BLKW + PADW + S]
                                        .broadcast_to([128, 128]))
                                else:
                                    off = it * BLKW + PADW + dlt * 128 + 511
                                    nc.sync.dma_start(
                                        cstrip[:, s * 128:(s + 1) * 128],
                                        AP(CPP[:].tensor, CPP[:].offset + off,
                                           [[WTOT - 1, 128], [1, 128]]))
                                # p2cT: diagonal DMA from BTP (already [j, i])
                                if dlt >= 5:       # i - j very negative -> idx 0
                                    nc.vector.tensor_copy(
                                        pstrip[:, s * 128:(s + 1) * 128],
                                        BTP[:, jt * BLKW + PADW:jt * BLKW + PADW + 1]
                                        .broadcast_to([128, 128]))
                                elif dlt <= -5:    # i - j large -> idx 1023
                                    nc.vector.tensor_copy(
                                        pstrip[:, s * 128:(s + 1) * 128],
                                        BTP[:, jt * BLKW + PADW + S - 1:jt * BLKW + PADW + S]
                                        .broadcast_to([128, 128]))
                                else:
                                    off = jt * BLKW + PADW + (it * 128 + 512 - jt * 128)
                                    nc.sync.dma_start(
                                        pstrip[:, s * 128:(s + 1) * 128],
                                        AP(BTP[:].tensor, BTP[:].offset + off,
                                           [[WTOT - 1, 128], [1, 128]]))
                            for s in range(4):
                                nc.tensor.matmul(
                                    psc[:, s * 128:(s + 1) * 128].bitcast(f32r),
                                    cstrip[:, s * 128:(s + 1) * 128],
                                    identR[:], is_transpose=True,
                                    start=False, stop=(s == 3))
                            tmp = strips.tile([128, 512], f32, tag="cstrip", name=f"tmp_{b}_{hl}_{ch}_{jt}")
                            nc.vector.tensor_tensor(
                                out=tmp[:], in0=psc[:], in1=pstrip[:], op=Add)
                            ex = strips.tile([128, 512], f32r, tag="ex")
                            nc.scalar.activation(
                                ex[:], tmp[:], Exp, bias=mbS[:, b * 8 + jt:b * 8 + jt + 1])
                            nc.tensor.matmul(
                                pov[0:65, :],
                                vext[:, (vbase + jt) * 65:(vbase + jt) * 65 + 65],
                                ex[:], start=(jt == 0), stop=(jt == 7))
                        ost = strips.tile([65, 512], f32, tag="fst",
                                          name=f"ost_{b}_{hl}_{ch}")
                        nc.scalar.copy(ost[:], pov[0:65, :])
                        nc.sync.dma_start(
                            OT[rg:rg + 64, ic0:ic0 + 512], ost[0:64, :])
                        with nc.allow_low_precision(reason="1/Z to f32r for PE bcast"):
                            nc.vector.reciprocal(
                                zrS[64:65, hl * S + ic0:hl * S + ic0 + 512],
                                ost[64:65, :])

                # ---- normalization + partial fc for batch b ----
                if STAGE < 4:
                    continue
                for ch in range(2):
                    ic0 = ch * 512
                    pz = psS.tile([128, 512], f32, tag="sc")
                    for hl in range(2):
                        nc.tensor.matmul(
                            pz[:], ohS[64:65, hl * 128:hl * 128 + 128],
                            zrS[64:65, hl * S + ic0:hl * S + ic0 + 512],
                            start=(hl == 0), stop=(hl == 1),
                            tile_position=(64, 0))
                    nc.vector.tensor_tensor(
                        out=OTn[:, ic0:ic0 + 512],
                        in0=OT[:, ic0:ic0 + 512],
                        in1=pz[:], op=Mult)
                for ft in range(8):
                    for ch in range(2):
                        pf = psO.tile([128, 512], f32, tag="ov")
                        nc.tensor.matmul(
                            pf[:], wcS[:, ft * 128:(ft + 1) * 128],
                            OTn[:, ch * 512:(ch + 1) * 512],
                            start=True, stop=True)
                        fst = strips.tile([128, 512], f32, tag="fst",
                                          name=f"fst_{b}_{ft}_{ch}")
                        nc.scalar.copy(fst[:], pf[:])
                        nc.sync.dma_start(
                            outT.ap()[ft * 128:(ft + 1) * 128,
                                      b * S + ch * 512:b * S + ch * 512 + 512],
                            fst[:])
            ctx_psO.__exit__(None, None, None)
            ctx_psS.__exit__(None, None, None)
    nc.compile()
    return nc


def _prep_in_maps(x, rel_pos_emb, padding_mask, Wq, bq, Wk, bk, Wv, bv,
                  Wqr, bqr, Wkr, bkr, Wc, bc):
    xf = np.ascontiguousarray(x.reshape(T, D), dtype=np.float32)
    rf = np.ascontiguousarray(rel_pos_emb.reshape(T, D), dtype=np.float32)
    mbias = np.where(padding_mask.astype(np.int64) == 1, np.float32(NEG),
                     np.float32(0.0)).astype(np.float32)  # [B, S]
    mbS = np.zeros((128, 16), np.float32)
    for b in range(B):
        for jt in range(8):
            mbS[:, b * 8 + jt] = mbias[b, jt * 128:(jt + 1) * 128]
    in_maps = []
    for c in range(NCORES):
        hs = slice(c * 128, (c + 1) * 128)
        m = {
            "xr": xf, "rr": rf, "mb": mbS,
            "wq": np.ascontiguousarray((Wq[hs] / SCALE).T),
            "wk": np.ascontiguousarray(Wk[hs].T),
            "wv": np.ascontiguousarray(Wv[hs].T),
            "wqr": np.ascontiguousarray((Wqr[hs] / SCALE).T),
            "wkr": np.ascontiguousarray(Wkr[hs].T),
            "wc": np.ascontiguousarray(Wc[:, hs].T),
            "bq": np.ascontiguousarray((bq[hs] / SCALE).reshape(128, 1)),
            "bk": np.ascontiguousarray(bk[hs].reshape(128, 1)),
            "bv": np.ascontiguousarray(bv[hs].reshape(128, 1)),
            "bqr": np.ascontiguousarray((bqr[hs] / SCALE).reshape(128, 1)),
            "bkr": np.ascontiguousarray(bkr[hs].reshape(128, 1)),
        }
        in_maps.append({k: np.asarray(v, dtype=np.float32) for k, v in m.items()})
    return in_maps


def kernel(**inputs):
    from concourse import bass_utils
    inputs = {k: np.asarray(v) for k, v in inputs.items()}
    in_maps = _prep_in_maps(**inputs)
    if "nc" not in _cached:
        _cached["nc"] = _build_nc()
    res = bass_utils.run_bass_kernel_spmd(
        _cached["nc"], in_maps, core_ids=list(range(NCORES)),
        **_cached.get("run_kwargs", {}))
    _cached["last_results"] = res
    acc = np.zeros((D, T), np.float32)
    for r in res.results:
        acc += r["outT"]
    out = acc.T.reshape(B, S, D) + inputs["bc"].astype(np.float32)
    return out.astype(np.float32)
